# revision 20
# baseline (speedup 1.0000x reference)
"""Bass/Trainium2 kernel for nn_HadamardClassifier.

Math: out = -scale * l2norm(x) @ H + bias, with H = H_16384[:2048, :14951]
(Sylvester). Since H_16384 = H_8 (x) H_2048 and rows < 2048 hit only row 0 of
the H_8 factor (all +1), H is just H_2048 tiled horizontally:
    out[:, j] = (x * (-scale/||x||_2)) @ H_2048[:, j % 2048] + bias[j]

Sharding: batch-parallel across 8 cores (512 rows each).

Precision budget (tolerance is max-abs-err / max|out| < 2e-2):
  - H_2048 entries +-1 are stored fp8_e4m3 (exact, halves the H load).
  - raw x rows are transposed on the PE straight off the f32 load and
    cast bf16 (~0.1% worst-case tail); the PE accepts mixed bf16 x fp8
    operands (verified exact on HW).
  - -scale/||row|| is applied on the per-partition scale input of the
    PSUM->SBUF copies (PSUM partitions = rows); the l2 norm is computed
    in parallel, off the matmul critical path.
  - Z and the output are bf16 (~0.2% of element each); host upcasts.

Schedule: pipelined transpose/matmul blocks keep the PE warm end to end
(a >3.4us idle re-throttles the HAM clock gate); matmuls run js-outer /
ic-inner so each 512-col slab drains while the next slab computes, and
stores are split in column halves per row-chunk so they flow on the
scalar HWDGE ring from ~20us on while loads ride the sync ring.
"""

import math

import numpy as np

B, IN, OUT = 4096, 2048, 14951
NCORES = 8
BLOC = B // NCORES  # 512
P = 128
PERIOD = 2048
NFULL = 7  # blocks covered by the main store views
TAIL = OUT - NFULL * PERIOD  # 615 = 512 (js0 blk7) + 103 (js1 blk7)
EPS = 1e-12
NCB = BLOC // P  # 4 batch chunks per core
NIC = IN // P  # 16 contraction chunks
NJS = PERIOD // 512  # 4 column slabs of 512
# bias packed js-major: per js, its 512-col piece of every block, 512-padded
NBLK_JS = [8, 8, 7, 7]  # js1's 8th seg is the 103-wide tail (padded)
OFF_JS = [0, 4096, 8192, 11776]
BIAS_PACK = 15360

_CACHE = {}
LAST_RESULT = None
PROFILE = False


def _build(scale_val: float):
    from contextlib import ExitStack

    import concourse.bass as bass
    import concourse.mybir as mybir
    import concourse.tile as tile
    from concourse import bacc, masks

    f32 = mybir.dt.float32
    bf16 = mybir.dt.bfloat16
    fp8 = mybir.dt.float8e4
    nc = bacc.Bacc("TRN2", target_bir_lowering=False, debug=False,
                   num_devices=NCORES)

    x_d = nc.dram_tensor("x", [BLOC, IN], f32, kind="ExternalInput")
    h_d = nc.dram_tensor("h", [IN, PERIOD], fp8, kind="ExternalInput")
    b_d = nc.dram_tensor("bias", [1, BIAS_PACK], bf16, kind="ExternalInput")
    # js2/js3 bias regions arrive pre-replicated across partitions: the
    # gpsimd partition_broadcast is too slow to cover them in time
    br_d = nc.dram_tensor("biasr", [P, BIAS_PACK - OFF_JS[2]], bf16,
                          kind="ExternalInput")
    o_d = nc.dram_tensor("out", [BLOC, OUT], bf16, kind="ExternalOutput")

    # [2048 rows] -> [p, ic] view so each SBUF partition p holds rows ic*128+p
    h_v = h_d[:, :].rearrange("(ic p) j -> p ic j", p=P)
    o_blk = o_d[:, 0 : NFULL * PERIOD].rearrange("r (blk c) -> r blk c",
                                                 c=PERIOD)

    with tile.TileContext(nc) as tc, ExitStack() as ctx:
        p_const = ctx.enter_context(tc.tile_pool(name="const", bufs=1))
        p_x = ctx.enter_context(tc.tile_pool(name="xload", bufs=NCB))
        p_w = ctx.enter_context(tc.tile_pool(name="work", bufs=1))
        p_ss = ctx.enter_context(tc.tile_pool(name="small", bufs=16))
        p_xq = ctx.enter_context(tc.tile_pool(name="xq", bufs=NCB))
        p_z = ctx.enter_context(tc.tile_pool(name="zsb", bufs=4))
        p_o = ctx.enter_context(tc.tile_pool(name="ostage", bufs=2))
        p_tl = ctx.enter_context(tc.tile_pool(name="tail", bufs=NCB))
        p_pst = ctx.enter_context(
            tc.tile_pool(name="psum_t", bufs=2, space="PSUM"))
        p_psz = ctx.enter_context(
            tc.tile_pool(name="psum_z", bufs=6, space="PSUM"))

        ident = p_const.tile([P, P], f32, tag="ident")
        masks.make_identity(nc, ident[:])

        # HAM warmup: open the PE clock gate before the real stream starts
        warm = p_pst.tile([P, 512], f32, tag="pst")
        for _ in range(12):
            nc.tensor.matmul(warm[:, 0:P], ident[:], ident[:], start=True,
                             stop=True)

        # loads on the sync ring, ordered to match consumption: x0 feeds the
        # first transpose block right away, then the H js-slabs pace the
        # first matmul chain, then the remaining x chunks; the scalar ring
        # is reserved for stores
        bias_rep = p_const.tile([P, BIAS_PACK], bf16, tag="bias_rep")
        nc.sync.dma_start(out=bias_rep[0:1, :], in_=b_d[:, :])
        # H laid out js-major: hs[p, js, ic, c] so each js slab is one DMA
        hs = p_const.tile([P, NJS, NIC, 512], fp8, tag="hs")
        xnats = []
        for cb in range(NCB):
            xnat = p_x.tile([P, IN], f32, tag="xnat", name=f"xnat{cb}")
            xnats.append(xnat)
        nc.sync.dma_start(out=xnats[0][:], in_=x_d[0:P, :])
        nc.sync.dma_start(out=hs[:, 0, :, :], in_=h_v[:, :, 0:512])
        nc.sync.dma_start(out=hs[:, 1, :, :], in_=h_v[:, :, 512:1024])
        nc.sync.dma_start(out=xnats[1][:], in_=x_d[P : 2 * P, :])
        nc.sync.dma_start(out=hs[:, 2, :, :], in_=h_v[:, :, 1024:1536])
        nc.sync.dma_start(out=hs[:, 3, :, :], in_=h_v[:, :, 1536:2048])
        nc.sync.dma_start(out=bias_rep[:, OFF_JS[2] :], in_=br_d[:, :])
        nc.sync.dma_start(out=xnats[2][:], in_=x_d[2 * P : 3 * P, :])
        nc.sync.dma_start(out=xnats[3][:], in_=x_d[3 * P : 4 * P, :])
        # js0/js1 bias regions broadcast on gpsimd, in consumption order
        for a in range(0, OFF_JS[2], 2048):
            nc.gpsimd.partition_broadcast(bias_rep[:, a : a + 2048],
                                          bias_rep[0:1, a : a + 2048])

        def phase1(cb):
            # transpose the RAW rows straight off the DMA; the l2 norm is
            # computed in parallel and -scale/||row|| is applied later on
            # the per-partition scale input of the PSUM->SBUF copies
            xnat = xnats[cb]
            xq = p_xq.tile([P, NIC, P], bf16, tag="xq")
            for g in range(4):  # groups of 4 transposes share one psum tile
                pst = p_pst.tile([P, 512], f32, tag="pst")
                for i in range(4):
                    ic = g * 4 + i
                    nc.tensor.transpose(pst[:, i * P : (i + 1) * P],
                                        xnat[:, ic * P : (ic + 1) * P],
                                        ident[:])
                dst = xq[:, g * 4 : (g + 1) * 4, :].rearrange(
                    "p i c -> p (i c)")
                if g % 2 == 0:
                    nc.scalar.copy(dst, pst[:])
                else:
                    nc.vector.tensor_copy(dst, pst[:])

            sq = p_w.tile([P, IN], bf16, tag="work")
            ss = p_ss.tile([P, 1], f32, tag="ss")
            nc.scalar.activation(sq[:], xnat[:],
                                 mybir.ActivationFunctionType.Square,
                                 accum_out=ss[:])
            nc.vector.tensor_scalar_max(ss[:], ss[:], EPS)
            nrm = p_ss.tile([P, 1], f32, tag="nrm")
            nc.scalar.sqrt(nrm[:], ss[:])
            inv = p_ss.tile([P, 1], f32, tag="inv")
            nc.vector.reciprocal(inv[:], nrm[:])
            mult = p_ss.tile([P, 1], f32, tag="mult")
            nc.vector.tensor_scalar_mul(mult[:], inv[:], -scale_val)
            return xq, mult

        def phase2(cb, xq, mult):
            r0 = cb * P
            ost = p_o.tile([P, NFULL, PERIOD], bf16, tag="ostage")
            tl = p_tl.tile([P, TAIL], bf16, tag="tail", name=f"tl{cb}")
            for js in range(NJS):
                c0 = js * 512
                boff = OFF_JS[js]
                psz = p_psz.tile([P, 512], f32, tag="psz")
                for ic in range(NIC):
                    nc.tensor.matmul(psz[:], xq[:, ic, :], hs[:, js, ic, :],
                                     start=(ic == 0), stop=(ic == NIC - 1))
                zsb = p_z.tile([P, 512], bf16, tag="zsb")
                if js % 2 == 0:
                    nc.scalar.mul(zsb[:], psz[:], mult[:, 0:1])
                else:
                    nc.vector.tensor_scalar_mul(zsb[:], psz[:], mult[:, 0:1])

                zb4 = zsb[:, :].unsqueeze(1).broadcast_to((P, 4, 512))
                zb3 = zsb[:, :].unsqueeze(1).broadcast_to((P, 3, 512))
                nc.vector.tensor_add(
                    ost[:, 0:4, c0 : c0 + 512], zb4,
                    bias_rep[:, boff : boff + 2048].rearrange(
                        "p (b c) -> p b c", b=4))
                nc.vector.tensor_add(
                    ost[:, 4:7, c0 : c0 + 512], zb3,
                    bias_rep[:, boff + 2048 : boff + 3584].rearrange(
                        "p (b c) -> p b c", b=3))
                if js == 0:
                    nc.vector.tensor_add(
                        tl[:, 0:512], zsb[:, :],
                        bias_rep[:, boff + 3584 : boff + 4096])
                elif js == 1:
                    nc.vector.tensor_add(
                        tl[:, 512:615], zsb[:, 0:103],
                        bias_rep[:, boff + 3584 : boff + 3687])
                    nc.scalar.dma_start(
                        out=o_d[r0 : r0 + P, NFULL * PERIOD : OUT],
                        in_=tl[:, :])
                last = cb == NCB - 1
                if last:
                    # per-slab stores so the very last store is small
                    nc.scalar.dma_start(
                        out=o_blk[r0 : r0 + P, :, c0 : c0 + 512],
                        in_=ost[:, :, c0 : c0 + 512])
                elif js == 1:
                    nc.scalar.dma_start(
                        out=o_blk[r0 : r0 + P, :, 0:1024],
                        in_=ost[:, :, 0:1024])
                elif js == 3:
                    nc.scalar.dma_start(
                        out=o_blk[r0 : r0 + P, :, 1024:2048],
                        in_=ost[:, :, 1024:2048])

        # software pipeline: phase1(cb+1) is emitted before phase2(cb) so the
        # PE queue alternates transpose and matmul blocks with no dead gaps
        # (a >3.4us PE idle re-throttles the HAM clock gate to half rate)
        prev = phase1(0)
        for cb in range(1, NCB):
            cur = phase1(cb)
            phase2(cb - 1, *prev)
            prev = cur
        phase2(NCB - 1, *prev)

    nc.compile()
    return nc


def _pack_bias(bias: np.ndarray) -> np.ndarray:
    import ml_dtypes
    pack = np.zeros((1, BIAS_PACK), dtype=np.float32)
    for js in range(NJS):
        for blk in range(NBLK_JS[js]):
            src0 = blk * PERIOD + js * 512
            seg = bias[src0 : src0 + 512]
            pack[0, OFF_JS[js] + blk * 512 :
                 OFF_JS[js] + blk * 512 + len(seg)] = seg
    return pack.astype(ml_dtypes.bfloat16)


def kernel(x, hadamard, scale, bias):
    global LAST_RESULT
    import ml_dtypes
    from concourse.bass_utils import run_bass_kernel_spmd

    x = np.ascontiguousarray(np.asarray(x, dtype=np.float32))
    hadamard = np.asarray(hadamard, dtype=np.float32)
    bias = np.asarray(bias, dtype=np.float32)
    scale_val = float(np.asarray(scale).reshape(-1)[0])

    h2 = np.ascontiguousarray(hadamard[:, :PERIOD])
    # the whole kernel rests on the 2048-periodicity of the weight columns
    for k in range(1, NFULL):
        assert np.array_equal(hadamard[:, k * PERIOD : (k + 1) * PERIOD], h2), (
            "hadamard is not 2048-periodic; kernel assumption violated")
    assert np.array_equal(hadamard[:, NFULL * PERIOD :], h2[:, :TAIL])
    h8 = h2.astype(ml_dtypes.float8_e4m3)
    assert np.array_equal(h8.astype(np.float32), h2), "H not fp8-exact"

    key = scale_val
    if key not in _CACHE:
        _CACHE[key] = _build(scale_val)
    nc = _CACHE[key]

    bias_pack = _pack_bias(bias)
    bias_rep23 = np.ascontiguousarray(
        np.broadcast_to(bias_pack[0:1, OFF_JS[2] :],
                        (P, BIAS_PACK - OFF_JS[2])))
    in_maps = [
        {"x": np.ascontiguousarray(x[c * BLOC : (c + 1) * BLOC]),
         "h": h8, "bias": bias_pack, "biasr": bias_rep23}
        for c in range(NCORES)
    ]
    res = run_bass_kernel_spmd(nc, in_maps, list(range(NCORES)),
                               trace=PROFILE)
    LAST_RESULT = res
    out = np.concatenate(
        [res.results[c]["out"].astype(np.float32) for c in range(NCORES)],
        axis=0)
    return out


# revision 21
# speedup vs baseline: 1.2992x; 1.2992x over previous
"""Bass/Trainium2 kernel for nn_HadamardClassifier.

Math: out = -scale * l2norm(x) @ H + bias, with H = H_16384[:2048, :14951]
(Sylvester). Since H_16384 = H_8 (x) H_2048 and rows < 2048 hit only row 0 of
the H_8 factor (all +1), H is just H_2048 tiled horizontally:
    out[:, j] = (x * (-scale/||x||_2)) @ H_2048[:, j % 2048] + bias[j]

Sharding: batch-parallel across 8 cores (512 rows each).

Precision budget (tolerance is max-abs-err / max|out| < 2e-2):
  - H_2048 entries +-1 are stored fp8_e4m3 (exact, halves the H load).
  - raw x rows are transposed on the PE straight off the f32 load and
    cast bf16 (~0.1% worst-case tail); the PE accepts mixed bf16 x fp8
    operands (verified exact on HW).
  - -scale/||row|| is applied on the per-partition scale input of the
    PSUM->SBUF copies (PSUM partitions = rows); the l2 norm is computed
    in parallel, off the matmul critical path.
  - Z and the output are bf16 (~0.2% of element each); host upcasts.

Schedule notes (hardware-measured):
  - pipelined transpose/matmul blocks keep the PE warm end to end (a
    >3.4us idle re-throttles the HAM clock gate to half rate);
  - stores must be large contiguous per-partition runs: the 28KB/row
    full-width store sustains ~350GB/s, 2KB segments are OK, 1KB
    segments collapse to ~140GB/s;
  - gpsimd partition_broadcast is slow (~4.5us per 2048 cols), so only
    the early-needed js0/js1 bias regions use it; js2/js3 arrive
    pre-replicated from DRAM during an otherwise idle DMA window.
"""

import math

import numpy as np

B, IN, OUT = 4096, 2048, 14951
NCORES = 8
BLOC = B // NCORES  # 512
P = 128
PERIOD = 2048
NFULL = 7  # blocks covered by the main store views
TAIL = OUT - NFULL * PERIOD  # 615 = 512 (js0 blk7) + 103 (js1 blk7)
EPS = 1e-12
NCB = BLOC // P  # 4 batch chunks per core
NIC = IN // P  # 16 contraction chunks
NJS = PERIOD // 512  # 4 column slabs of 512
# bias packed js-major: per js, its 512-col piece of every block, 512-padded
NBLK_JS = [8, 8, 7, 7]  # js1's 8th seg is the 103-wide tail (padded)
OFF_JS = [0, 4096, 8192, 11776]
BIAS_PACK = 15360

_CACHE = {}
LAST_RESULT = None
PROFILE = False


def _build(scale_val: float):
    from contextlib import ExitStack

    import concourse.bass as bass
    import concourse.mybir as mybir
    import concourse.tile as tile
    from concourse import bacc, masks

    f32 = mybir.dt.float32
    bf16 = mybir.dt.bfloat16
    fp8 = mybir.dt.float8e4
    nc = bacc.Bacc("TRN2", target_bir_lowering=False, debug=False,
                   num_devices=NCORES)

    x_d = nc.dram_tensor("x", [BLOC, IN], f32, kind="ExternalInput")
    h_d = nc.dram_tensor("h", [IN, PERIOD], fp8, kind="ExternalInput")
    b_d = nc.dram_tensor("bias", [1, BIAS_PACK], bf16, kind="ExternalInput")
    br_d = nc.dram_tensor("biasr", [P, BIAS_PACK - OFF_JS[2]], bf16,
                          kind="ExternalInput")
    o_d = nc.dram_tensor("out", [BLOC, OUT], bf16, kind="ExternalOutput")

    # [2048 rows] -> [p, ic] view so each SBUF partition p holds rows ic*128+p
    h_v = h_d[:, :].rearrange("(ic p) j -> p ic j", p=P)
    o_blk = o_d[:, 0 : NFULL * PERIOD].rearrange("r (blk c) -> r blk c",
                                                 c=PERIOD)

    with tile.TileContext(nc) as tc, ExitStack() as ctx:
        p_const = ctx.enter_context(tc.tile_pool(name="const", bufs=1))
        p_x = ctx.enter_context(tc.tile_pool(name="xload", bufs=NCB))
        p_w = ctx.enter_context(tc.tile_pool(name="work", bufs=1))
        p_ss = ctx.enter_context(tc.tile_pool(name="small", bufs=16))
        p_xq = ctx.enter_context(tc.tile_pool(name="xq", bufs=NCB))
        p_z = ctx.enter_context(tc.tile_pool(name="zsb", bufs=4))
        p_o = ctx.enter_context(tc.tile_pool(name="ostage", bufs=2))
        p_tl = ctx.enter_context(tc.tile_pool(name="tail", bufs=NCB))
        p_pst = ctx.enter_context(
            tc.tile_pool(name="psum_t", bufs=2, space="PSUM"))
        p_psz = ctx.enter_context(
            tc.tile_pool(name="psum_z", bufs=6, space="PSUM"))

        ident = p_const.tile([P, P], f32, tag="ident")
        masks.make_identity(nc, ident[:])

        # HAM warmup: open the PE clock gate before the real stream starts
        warm = p_pst.tile([P, 512], f32, tag="pst")
        for _ in range(24):
            nc.tensor.matmul(warm[:, 0:P], ident[:], ident[:], start=True,
                             stop=True)

        # loads on the sync ring: x chunks interleaved with H ic-groups (the
        # ic-outer matmul order consumes H progressively); the pre-replicated
        # js2/js3 bias rides the post-load DMA lull; scalar ring = stores
        bias_rep = p_const.tile([P, BIAS_PACK], bf16, tag="bias_rep")
        nc.sync.dma_start(out=bias_rep[0:1, :], in_=b_d[:, :])
        hs = p_const.tile([P, NIC, PERIOD], fp8, tag="hs")
        xnats = []
        for cb in range(NCB):
            xnat = p_x.tile([P, IN], f32, tag="xnat", name=f"xnat{cb}")
            nc.sync.dma_start(out=xnat[:], in_=x_d[cb * P : (cb + 1) * P, :])
            xnats.append(xnat)
            nc.sync.dma_start(out=hs[:, cb * 4 : (cb + 1) * 4, :],
                              in_=h_v[:, cb * 4 : (cb + 1) * 4, :])
        nc.sync.dma_start(out=bias_rep[:, OFF_JS[2] :], in_=br_d[:, :])
        # js0/js1 bias regions broadcast on gpsimd, in consumption order
        for a in range(0, OFF_JS[2], 2048):
            nc.gpsimd.partition_broadcast(bias_rep[:, a : a + 2048],
                                          bias_rep[0:1, a : a + 2048])

        def phase1(cb):
            # transpose the RAW rows straight off the DMA; the l2 norm is
            # computed in parallel and -scale/||row|| is applied later on
            # the per-partition scale input of the PSUM->SBUF copies
            xnat = xnats[cb]
            xq = p_xq.tile([P, NIC, P], bf16, tag="xq")
            for g in range(4):  # groups of 4 transposes share one psum tile
                pst = p_pst.tile([P, 512], f32, tag="pst")
                for i in range(4):
                    ic = g * 4 + i
                    nc.tensor.transpose(pst[:, i * P : (i + 1) * P],
                                        xnat[:, ic * P : (ic + 1) * P],
                                        ident[:])
                dst = xq[:, g * 4 : (g + 1) * 4, :].rearrange(
                    "p i c -> p (i c)")
                if g % 2 == 0:
                    nc.scalar.copy(dst, pst[:])
                else:
                    nc.vector.tensor_copy(dst, pst[:])

            sq = p_w.tile([P, IN], bf16, tag="work")
            ss = p_ss.tile([P, 1], f32, tag="ss")
            nc.scalar.activation(sq[:], xnat[:],
                                 mybir.ActivationFunctionType.Square,
                                 accum_out=ss[:])
            nc.vector.tensor_scalar_max(ss[:], ss[:], EPS)
            nrm = p_ss.tile([P, 1], f32, tag="nrm")
            nc.scalar.sqrt(nrm[:], ss[:])
            inv = p_ss.tile([P, 1], f32, tag="inv")
            nc.vector.reciprocal(inv[:], nrm[:])
            mult = p_ss.tile([P, 1], f32, tag="mult")
            nc.vector.tensor_scalar_mul(mult[:], inv[:], -scale_val)
            return xq, mult

        def drain(cb, js, psz, mult, ost, tl):
            c0 = js * 512
            boff = OFF_JS[js]
            zsb = p_z.tile([P, 512], bf16, tag="zsb")
            if js % 2 == 0:
                nc.scalar.mul(zsb[:], psz[:], mult[:, 0:1])
            else:
                nc.vector.tensor_scalar_mul(zsb[:], psz[:], mult[:, 0:1])
            zb4 = zsb[:, :].unsqueeze(1).broadcast_to((P, 4, 512))
            zb3 = zsb[:, :].unsqueeze(1).broadcast_to((P, 3, 512))
            nc.vector.tensor_add(
                ost[:, 0:4, c0 : c0 + 512], zb4,
                bias_rep[:, boff : boff + 2048].rearrange(
                    "p (b c) -> p b c", b=4))
            nc.vector.tensor_add(
                ost[:, 4:7, c0 : c0 + 512], zb3,
                bias_rep[:, boff + 2048 : boff + 3584].rearrange(
                    "p (b c) -> p b c", b=3))
            if js == 0:
                nc.vector.tensor_add(
                    tl[:, 0:512], zsb[:, :],
                    bias_rep[:, boff + 3584 : boff + 4096])
            elif js == 1:
                nc.vector.tensor_add(
                    tl[:, 512:615], zsb[:, 0:103],
                    bias_rep[:, boff + 3584 : boff + 3687])

        def phase2(cb, xq, mult):
            r0 = cb * P
            ost = p_o.tile([P, NFULL, PERIOD], bf16, tag="ostage")
            tl = p_tl.tile([P, TAIL], bf16, tag="tail", name=f"tl{cb}")
            if cb < NCB - 1:
                # ic-outer with a 4-bank psum fan: H is consumed in load
                # order and all four slabs complete together; the drains
                # overlap the next chunk's transpose/matmul blocks
                psz = [p_psz.tile([P, 512], f32, tag="psz", name=f"psz{js}")
                       for js in range(NJS)]
                for ic in range(NIC):
                    for js in range(NJS):
                        nc.tensor.matmul(psz[js][:], xq[:, ic, :],
                                         hs[:, ic, js * 512 : (js + 1) * 512],
                                         start=(ic == 0), stop=(ic == NIC - 1))
                for js in range(NJS):
                    drain(cb, js, psz[js], mult, ost, tl)
                eng = nc.scalar if cb % 2 else nc.sync
                eng.dma_start(out=o_d[r0 : r0 + P, NFULL * PERIOD : OUT],
                              in_=tl[:, :])
                # one contiguous 28 KB/partition store for blocks 0..6
                eng.dma_start(
                    out=o_d[r0 : r0 + P, 0 : NFULL * PERIOD],
                    in_=ost[:, :, :].rearrange("p b c -> p (b c)"))
            else:
                # last chunk: js-outer so each slab drains while the next
                # slab's matmuls run, and the store is split in two halves
                # (2KB segments) so the final store is half as long
                for js in range(NJS):
                    psz = p_psz.tile([P, 512], f32, tag="psz", name="pszl")
                    for ic in range(NIC):
                        nc.tensor.matmul(psz[:], xq[:, ic, :],
                                         hs[:, ic, js * 512 : (js + 1) * 512],
                                         start=(ic == 0), stop=(ic == NIC - 1))
                    drain(cb, js, psz, mult, ost, tl)
                    if js == 1:
                        nc.scalar.dma_start(
                            out=o_d[r0 : r0 + P, NFULL * PERIOD : OUT],
                            in_=tl[:, :])
                        nc.scalar.dma_start(
                            out=o_blk[r0 : r0 + P, :, 0:1024],
                            in_=ost[:, :, 0:1024])
                    elif js == 3:
                        nc.scalar.dma_start(
                            out=o_blk[r0 : r0 + P, :, 1024:2048],
                            in_=ost[:, :, 1024:2048])

        # software pipeline: phase1(cb+1) is emitted before phase2(cb) so the
        # PE queue alternates transpose and matmul blocks with no dead gaps
        # (a >3.4us PE idle re-throttles the HAM clock gate to half rate)
        prev = phase1(0)
        for cb in range(1, NCB):
            cur = phase1(cb)
            phase2(cb - 1, *prev)
            prev = cur
        phase2(NCB - 1, *prev)

    nc.compile()
    return nc


def _pack_bias(bias: np.ndarray) -> np.ndarray:
    import ml_dtypes
    pack = np.zeros((1, BIAS_PACK), dtype=np.float32)
    for js in range(NJS):
        for blk in range(NBLK_JS[js]):
            src0 = blk * PERIOD + js * 512
            seg = bias[src0 : src0 + 512]
            pack[0, OFF_JS[js] + blk * 512 :
                 OFF_JS[js] + blk * 512 + len(seg)] = seg
    return pack.astype(ml_dtypes.bfloat16)


def kernel(x, hadamard, scale, bias):
    global LAST_RESULT
    import ml_dtypes
    from concourse.bass_utils import run_bass_kernel_spmd

    x = np.ascontiguousarray(np.asarray(x, dtype=np.float32))
    hadamard = np.asarray(hadamard, dtype=np.float32)
    bias = np.asarray(bias, dtype=np.float32)
    scale_val = float(np.asarray(scale).reshape(-1)[0])

    h2 = np.ascontiguousarray(hadamard[:, :PERIOD])
    # the whole kernel rests on the 2048-periodicity of the weight columns
    for k in range(1, NFULL):
        assert np.array_equal(hadamard[:, k * PERIOD : (k + 1) * PERIOD], h2), (
            "hadamard is not 2048-periodic; kernel assumption violated")
    assert np.array_equal(hadamard[:, NFULL * PERIOD :], h2[:, :TAIL])
    h8 = h2.astype(ml_dtypes.float8_e4m3)
    assert np.array_equal(h8.astype(np.float32), h2), "H not fp8-exact"

    key = scale_val
    if key not in _CACHE:
        _CACHE[key] = _build(scale_val)
    nc = _CACHE[key]

    bias_pack = _pack_bias(bias)
    bias_rep23 = np.ascontiguousarray(
        np.broadcast_to(bias_pack[0:1, OFF_JS[2] :],
                        (P, BIAS_PACK - OFF_JS[2])))
    in_maps = [
        {"x": np.ascontiguousarray(x[c * BLOC : (c + 1) * BLOC]),
         "h": h8, "bias": bias_pack, "biasr": bias_rep23}
        for c in range(NCORES)
    ]
    res = run_bass_kernel_spmd(nc, in_maps, list(range(NCORES)),
                               trace=PROFILE)
    LAST_RESULT = res
    out = np.concatenate(
        [res.results[c]["out"].astype(np.float32) for c in range(NCORES)],
        axis=0)
    return out
